# revision 1
# baseline (speedup 1.0000x reference)
"""Tensor-parallel causal self-attention (GQA + RoPE) on one TRN2 chip (8 NeuronCores).

Megatron-style TP over heads: core i computes q-heads {2i, 2i+1} (kv head i//2),
runs blocked causal attention for those heads entirely on-core, then the partial
c_proj  y_i @ Wo[rows_i, :].  The 8 partial [T, C] outputs are summed on the host
(the TP all-reduce), which is pure gather/unshard data movement.

Layout (contraction dims on SBUF partitions):
  xT   [128, NCT, T] tile-major (host pre-arranged, bf16) so wide strided DMAs
      stay legal; first halves stream per-tile ahead of the 8-chain group.
  qT/kT = Wq/Wk-proj emitted directly as [HD, T] via lhsT=W, rhs=xT
  RoPE rotate_half runs on PE as a +-1 permutation matmul; split into raw-copy
      (ACT/DVE) and rot-matmul phases so chain PSUM frees early.
  scoresT [s,t]: two 128-s-tiles per [128,1024] PSUM strip (2 banks); ONE wide
      exp per strip on ACT (amortizes activation overhead); diagonal strips
      exp only the live causal ranges.
  softmax denominator: DVE accumulates pt strips elementwise in bf16, then
      per-(h,t-block) gpsimd partition_all_reduce (reduce+broadcast in one op),
      pipelined in halves with the DVE reciprocal to cut block-boundary
      latency; normalization folds into the PSUM->SBUF eviction multiply.
      No PE cycles are spent on the denominator.
  v natural [s,d] from a vT projection + DMA-transpose (N=512 matmuls)
  outT [d,t] += pv accumulation with lhsT=v_nat, rhs=pt strip slices
  c_proj (m,n) items drain from a work-queue interleaved into attention blocks
      to fill PE stalls while ACT runs exp; evictions alternate ACT/DVE
      (Pool cannot touch PSUM); finale items rotate through the idle score
      strips' PSUM banks; the last row DMAs out in chunks for a short tail.

Schedule: 8-chain ci-interleaved group (t-blocks 0+1) races the input DMA
stream; t-blocks 2+3 project as half-blocks from a PE fill queue interleaved
between attention strips, so attn0/attn1's exp/softmax work rides inside the
projection phase.  Causal masking: off-diagonal s-tiles need no mask; the 4
diagonal s-tiles per t-block are computed at narrowed width with a -1e30
additive mask for the intra-tile triangle.
"""

import math
from contextlib import ExitStack

import ml_dtypes
import numpy as np

import concourse.bass as bass
import concourse.tile as tile
from concourse import bacc, mybir, bass_isa
from concourse.bass import ts, ds
from concourse.bass_utils import run_bass_kernel_spmd

# ---------------- problem constants (hardcoded per contest rules) ------------
B, T, C = 1, 2048, 2048
H, KH, HD = 16, 4, 128
NCORES = 8
HQ = H // NCORES            # 2 query heads per core
ROPE_BASE = 10000.0
SCALE = 1.0 / math.sqrt(HD)
TB = 512                    # t-block (moving free dim) for attention
NT = T // TB                # 4
NCT = C // 128              # 16 contraction tiles for projections
NS = T // 128               # 16 key/value s-tiles
BF16 = mybir.dt.bfloat16
F32 = mybir.dt.float32
EXPF = mybir.ActivationFunctionType.Exp
NEG = -1.0e30

_NC_CACHE = {}


def _bf16(a):
    return np.ascontiguousarray(np.asarray(a, dtype=np.float32).astype(ml_dtypes.bfloat16))


def _emit(tc, dr, out_d):
    nc = tc.nc
    with ExitStack() as ctx:
        def sb(name, bufs):
            return ctx.enter_context(tc.tile_pool(name=name, bufs=bufs))

        def ps(name, bufs):
            return ctx.enter_context(tc.tile_pool(name=name, bufs=bufs, space="PSUM"))

        p_xt = sb("xt", 1)
        p_wq = sb("wq", 1)
        p_wo = sb("wo", 1)
        p_trig = sb("trig", 1)
        p_mask = sb("mask", 1)
        p_junk = sb("junk", 1)
        p_qt = sb("qt", HQ)
        p_kt = sb("kt", 1)
        p_v = sb("v", NS)
        p_vt = sb("vt", 2)
        p_yt = sb("yt", HQ)
        p_qraw = sb("qraw", 8)
        p_rtmp = sb("rtmp", 6)
        p_pt = sb("pt", 6)          # [128,1024] bf16 exp strips
        p_acc = sb("acc", 2)        # [128,TB] bf16 denominator partial sums
        p_rb = sb("rb", 3)          # [128,TB] f32 partition_all_reduce out
        p_ri = sb("ri", 2)          # [128,TB] f32 reciprocal out
        p_stage = sb("stage", 4)
        ps_strip = ps("ps_strip", 2)  # [128,1024] score strips (+rope rot)
        ps_acc = ps("ps_acc", 2)      # [128,TB] attention outT accumulators
        ps_cp = ps("ps_cp", 2)        # [128,TB] c_proj / proj k,v chains

        # ---------------- input loads (HWDGE queues: sync + scalar only; the
        # gpsimd queue is software-DGE and burns ~1us of Pool engine time per
        # descriptor, delaying rope adds / evictions) -------------------------
        wqkv_all = p_wq.tile([128, NCT, 4 * HD], BF16, name="wqkv", tag="wq")
        xt_all = p_xt.tile([128, NCT, T], BF16, name="xta", tag="xt")
        xt = [xt_all[:, i, :] for i in range(NCT)]
        # The 8-chain group (t-blocks 0+1) only reads xt[:, 0:1024]: stream all
        # first-halves at the head of the sync queue (delivery ~1.1us/tile vs
        # ~1.7us/tile consumption); second halves land as one strided DMA
        # behind them (merged: the serial HWDGE descriptor processor is near
        # saturation during the input phase).
        nc.sync.dma_start(wqkv_all[:, 0, :], dr["wqkv"][:, 0, :])
        nc.scalar.dma_start(xt_all[:, 0, 0:512], dr["xt"][:, 0, 0:512])
        nc.sync.dma_start(xt_all[:, 0, 512:1024], dr["xt"][:, 0, 512:1024])
        for i in range(1, NCT):
            nc.sync.dma_start(xt_all[:, i, 0:1024], dr["xt"][:, i, 0:1024])
        for i in range(0, NCT, 4):   # second halves: 4-tile chunks (a single
            nc.sync.dma_start(xt_all[:, i:i + 4, 1024:T],   # 12us transfer
                              dr["xt"][:, i:i + 4, 1024:T])  # would hog the wire)
        nc.scalar.dma_start(wqkv_all[:, 1, :], dr["wqkv"][:, 1, :])
        nc.scalar.dma_start(wqkv_all[:, 2, :], dr["wqkv"][:, 2, :])
        nc.scalar.dma_start(wqkv_all[:, 3:5, :], dr["wqkv"][:, 3:5, :])
        nc.scalar.dma_start(wqkv_all[:, 5:8, :], dr["wqkv"][:, 5:8, :])
        wq = [wqkv_all[:, i, :] for i in range(NCT)]  # cols [0, 256) = Wq
        wk = [wqkv_all[:, i, 256:384] for i in range(NCT)]
        wv = [wqkv_all[:, i, 384:512] for i in range(NCT)]
        # trig first-halves + rot/mask tables land mid-stream (needed by the
        # grp rope evictions at ~31us); wqkv tail keeps pace with the group.
        trig = p_trig.tile([128, 2 * T], BF16, name="trig", tag="trig")
        cost = trig[:, 0:T]
        sint = trig[:, T:2 * T]
        nc.scalar.dma_start(trig[:, 0:1024], dr["trig"][:, 0:1024])
        nc.scalar.dma_start(trig[:, T:T + 1024], dr["trig"][:, T:T + 1024])
        rmat = p_mask.tile([128, 128], BF16, name="rmat", tag="rmat")
        nc.scalar.dma_start(rmat[:], dr["rmat"][:, :])
        masks = p_mask.tile([128, 128], F32, name="masks", tag="mask")
        nc.scalar.dma_start(masks[:], dr["masks"][:, :])
        nc.scalar.dma_start(wqkv_all[:, 8:12, :], dr["wqkv"][:, 8:12, :])
        nc.scalar.dma_start(wqkv_all[:, 12:NCT, :], dr["wqkv"][:, 12:NCT, :])
        nc.scalar.dma_start(trig[:, 1024:T], dr["trig"][:, 1024:T])
        nc.scalar.dma_start(trig[:, T + 1024:2 * T], dr["trig"][:, T + 1024:2 * T])
        wo2 = p_wo.tile([128, HQ * C], BF16, name="wo2", tag="wo")
        wo = [wo2[:, ts(h, C)] for h in range(HQ)]
        nc.scalar.dma_start(wo2[:, 0:C], dr["wo"][:, 0:C])
        nc.scalar.dma_start(wo2[:, C:2 * C], dr["wo"][:, C:2 * C])
        # preload exp table while DMAs run (after the scalar-queue dma issues
        # so LoadActFuncSet doesn't block them)
        junk = p_junk.tile([128, 512], BF16, name="junk", tag="junk")
        nc.vector.memset(junk[0:1, 0:1], 0.0)
        nc.scalar.activation(junk[0:1, 0:1], junk[0:1, 0:1], EXPF, scale=1.0)

        # ---------------- rope helpers (two-phase) ----------------
        def raw_phase(psum, eng=None):
            """chain psum [128(d), TB] f32 -> raw bf16 SBUF copy (ACT/DVE)."""
            raw = p_qraw.tile([128, TB], BF16, name="rraw", tag="qraw")
            if eng is nc.vector:
                nc.vector.tensor_copy(raw[:], psum[:])
            else:
                nc.scalar.copy(raw[:], psum[:])
            return raw

        rope_add_rr = [0]

        def rot_phase(raw, bt, dst):
            """raw [128, TB] bf16 -> RoPE -> dst (bf16 slice [128, TB])."""
            cs = cost[:, ts(bt, TB)]
            sn = sint[:, ts(bt, TB)]
            rot = ps_strip.tile([128, TB], F32, name="rot", tag="strip")
            nc.tensor.matmul(rot[:], lhsT=rmat[:], rhs=raw[:], start=True, stop=True)
            t1 = p_rtmp.tile([128, TB], BF16, name="rt1", tag="rtmp")
            t2 = p_rtmp.tile([128, TB], BF16, name="rt2", tag="rtmp")
            nc.vector.tensor_mul(t1[:], raw[:], cs)
            nc.vector.tensor_mul(t2[:], rot[:], sn)
            rope_add_rr[0] += 1
            eng = nc.gpsimd if rope_add_rr[0] % 2 else nc.vector
            eng.tensor_add(dst, t1[:], t2[:])

        qT = [p_qt.tile([128, T], BF16, name=f"qT{h}", tag="qt") for h in range(HQ)]
        kT = p_kt.tile([128, T], BF16, name="kT", tag="kt")
        v = [p_v.tile([128, HD], BF16, name=f"v{s}", tag="v") for s in range(NS)]
        yT = [p_yt.tile([128, T], BF16, name=f"yT{h}", tag="yt") for h in range(HQ)]
        dma_rr = [nc.sync, nc.scalar]

        def v_evict(pv, bt):
            vts = p_vt.tile([128, TB], BF16, name="vts", tag="vt")
            nc.vector.tensor_copy(vts[:], pv[:])
            for j in range(TB // 128):   # sync queue only: keep ACT.SEQ clear
                nc.sync.dma_start_transpose(v[4 * bt + j][:], vts[:, ts(j, 128)])

        # ---------------- c_proj work queue ----------------
        cp_queue = []          # (m, n) items ready to emit
        st_tiles = {}
        evict_rr = [0]
        last_row = [-1]
        fin_strip = [None]

        def emit_cp_item(finale=False):
            if not cp_queue:
                return False
            m, n = cp_queue.pop(0)
            if n == 0:
                st_tiles[m] = p_stage.tile([128, C], BF16, name="st", tag="stage")
            st = st_tiles[m]
            if finale and evict_rr[0] % 2 == 0:
                # strips are idle in the finale: rotate c_proj psum through
                # them too, so items never wait on the 2 cp banks
                if fin_strip[0] is None:
                    fin_strip[0] = ps_strip.tile([128, 1024], F32, name="fs",
                                                 tag="strip")
                    pc = fin_strip[0][:, 0:512]
                else:
                    pc = fin_strip[0][:, 512:1024]
                    fin_strip[0] = None
            else:
                pc = ps_cp.tile([128, TB], F32, name="pc", tag="cp")
            for h in range(HQ):
                nc.tensor.matmul(
                    pc[:], lhsT=yT[h][:, ts(m, 128)], rhs=wo[h][:, ts(n, TB)],
                    start=(h == 0), stop=(h == HQ - 1))
            e = evict_rr[0] % 2   # Pool/GpSimd cannot read PSUM
            evict_rr[0] += 1
            if e == 0:
                nc.scalar.copy(st[:, ts(n, TB)], pc[:])
            else:
                nc.vector.tensor_copy(st[:, ts(n, TB)], pc[:])
            if m == last_row[0]:        # last row: chunked DMA, short tail
                dma_rr[evict_rr[0] % 2].dma_start(out_d[ts(m, 128), ts(n, TB)],
                                                  st[:, ts(n, TB)])
            elif finale and n % 2 == 1:  # finale rows: half-row DMAs issue
                dma_rr[m % 2].dma_start(   # earlier, evening out the wire
                    out_d[ts(m, 128), ds((n - 1) * TB, 2 * TB)],
                    st[:, ds((n - 1) * TB, 2 * TB)])
            elif n == 3:
                dma_rr[m % 2].dma_start(out_d[ts(m, 128), :], st[:])
            return True

        # ---------------- PE fill queue (proj half-blocks, then cp items) ----
        fill_q = []            # (tag, thunk)

        def run_fill(k):
            while k > 0 and fill_q:
                fill_q.pop(0)[1]()
                k -= 1
            while k > 0 and emit_cp_item():
                k -= 1

        def drain_fill_until(tag):
            while fill_q and fill_q[0][0] <= tag:
                fill_q.pop(0)[1]()

        # ---------------- attention ----------------
        def attn_block(bt, fill=0):
            nbs = 4 * (bt + 1)
            for h in range(HQ):
                po = ps_acc.tile([128, TB], F32, name="po", tag="acc")
                acc = p_acc.tile([128, TB], BF16, name="dacc", tag="acc")
                prev = None
                for k in range(nbs // 2):
                    b0, b1 = 2 * k, 2 * k + 1
                    strip = ps_strip.tile([128, 1024], F32, name="sc", tag="strip")
                    pt = p_pt.tile([128, 1024], BF16, name="pt", tag="pt")
                    offs = []
                    for si, bs in enumerate((b0, b1)):
                        j = bs - 4 * bt
                        off = max(j, 0) * 128
                        offs.append(off)
                        nc.tensor.matmul(
                            strip[:, 512 * si + off:512 * (si + 1)],
                            lhsT=kT[:, ts(bs, 128)],
                            rhs=qT[h][:, ds(bt * TB + off, TB - off)],
                            start=True, stop=True)
                        if j >= 0:   # diagonal: intra-tile causal triangle
                            nc.vector.tensor_add(
                                strip[:, ds(512 * si + off, 128)],
                                strip[:, ds(512 * si + off, 128)],
                                masks[:, :])
                    if offs[1] == 0:
                        nc.scalar.activation(pt[:], strip[:], EXPF, scale=SCALE)
                    else:   # diagonal strips: exp only the live ranges
                        nc.scalar.activation(pt[:, offs[0]:512],
                                             strip[:, offs[0]:512],
                                             EXPF, scale=SCALE)
                        nc.scalar.activation(pt[:, 512 + offs[1]:1024],
                                             strip[:, 512 + offs[1]:1024],
                                             EXPF, scale=SCALE)
                    # denominator partial sums (DVE, partition lanes)
                    if k == 0:
                        nc.vector.tensor_copy(acc[:], pt[:, 0:512])
                    else:
                        nc.vector.tensor_add(
                            acc[:, offs[0]:512], acc[:, offs[0]:512],
                            pt[:, offs[0]:512])
                    nc.vector.tensor_add(
                        acc[:, offs[1]:512], acc[:, offs[1]:512],
                        pt[:, 512 + offs[1]:1024])
                    run_fill(fill)
                    if prev is not None:
                        pv_emit(po, bt, nbs, *prev)
                    prev = (pt, b0, b1, offs)
                pv_emit(po, bt, nbs, *prev)
                # normalize: partition all-reduce -> reciprocal -> fold into
                # the PSUM->SBUF eviction multiply
                rb = p_rb.tile([128, TB], F32, name="rb", tag="rb")
                ri = p_ri.tile([128, TB], F32, name="ri", tag="ri")
                for half in range(2):   # halves pipeline Pool->DVE: shorter
                    hs = ds(half * 256, 256)        # block-boundary latency
                    nc.gpsimd.partition_all_reduce(
                        rb[:, hs], acc[:, hs], channels=128,
                        reduce_op=bass_isa.ReduceOp.add)
                    nc.vector.reciprocal(ri[:, hs], rb[:, hs])
                    for e in range(2):
                        c = ds(bt * TB + half * 256 + e * 128, 128)
                        cp = ds(half * 256 + e * 128, 128)
                        nc.vector.tensor_mul(yT[h][:, c], po[:, cp], ri[:, cp])
            cp_queue.extend((4 * bt + sub, n)
                            for sub in range(NT) for n in range(C // TB))

        def pv_emit(po, bt, nbs, pt, b0, b1, offs):
            for si, bs in enumerate((b0, b1)):
                off = offs[si]
                nc.tensor.matmul(
                    po[:, off:TB], lhsT=v[bs][:],
                    rhs=pt[:, 512 * si + off:512 * (si + 1)],
                    start=(bs == 0), stop=(bs == nbs - 1))

        # -------- projections --------
        # t-blocks 0+1: one ci-interleaved 8-chain group across all 8 PSUM
        # banks — maximizes PE progress while x is still streaming in.
        s0 = ps_strip.tile([128, 1024], F32, name="gs0", tag="strip")
        s1 = ps_strip.tile([128, 1024], F32, name="gs1", tag="strip")
        a0 = ps_acc.tile([128, TB], F32, name="ga0", tag="acc")
        a1 = ps_acc.tile([128, TB], F32, name="ga1", tag="acc")
        c0 = ps_cp.tile([128, TB], F32, name="gc0", tag="cp")
        c1 = ps_cp.tile([128, TB], F32, name="gc1", tag="cp")
        grp = [s0[:, 0:512], s0[:, 512:1024], s1[:, 0:512], s1[:, 512:1024],
               a0[:], a1[:], c0[:], c1[:]]
        specs4 = [(wq, 0), (wq, 1), (wk, None), (wv, None)]
        for ci in range(NCT):
            for i in range(8):
                b = i // 4
                w, hh = specs4[i % 4]
                lhsT = w[ci][:, ts(hh, HD)] if hh is not None else w[ci]
                nc.tensor.matmul(
                    grp[i], lhsT=lhsT, rhs=xt[ci][:, ts(b, TB)],
                    start=(ci == 0), stop=(ci == NCT - 1))
        # rope raw copies split ACT/DVE (k first: attn0 needs kT earliest);
        # chain PSUM frees as soon as its raw lands.
        raw_k0 = raw_phase(grp[2], nc.scalar)
        raw_q00 = raw_phase(grp[0], nc.vector)
        raw_q10 = raw_phase(grp[1], nc.scalar)
        v_evict(grp[3], 0)
        raw_k1 = raw_phase(grp[6], nc.vector)
        raw_q01 = raw_phase(grp[4], nc.scalar)
        raw_q11 = raw_phase(grp[5], nc.vector)
        v_evict(grp[7], 1)

        # proj half-blocks for t-blocks 2+3 go into the PE fill queue
        def make_proj_half(bt, kinds, dsts, tag):
            box = {}

            def step(ci):
                def thunk():
                    if "tiles" not in box:
                        box["tiles"] = [ps_cp.tile([128, TB], F32, name="ph",
                                                   tag="cp") for _ in kinds]
                    for t, (w, hh) in zip(box["tiles"], kinds):
                        lhsT = w[ci][:, ts(hh, HD)] if hh is not None else w[ci]

                        nc.tensor.matmul(
                            t[:], lhsT=lhsT, rhs=xt[ci][:, ts(bt, TB)],
                            start=(ci == 0), stop=(ci == NCT - 1))
                return thunk

            def finisher():
                engs = [nc.scalar, nc.vector]
                for i, (t, dst) in enumerate(zip(box["tiles"], dsts)):
                    if dst is None:
                        v_evict(t, bt)
                    else:
                        raw = raw_phase(t[:], engs[i % 2])
                        rot_phase(raw, bt, dst)

            for ci in range(NCT):
                fill_q.append((tag, step(ci)))
            fill_q.append((tag, finisher))

        make_proj_half(2, specs4[0:2], [qT[0][:, ts(2, TB)], qT[1][:, ts(2, TB)]], 1)
        make_proj_half(2, specs4[2:4], [kT[:, ts(2, TB)], None], 2)
        make_proj_half(3, specs4[0:2], [qT[0][:, ts(3, TB)], qT[1][:, ts(3, TB)]], 3)
        make_proj_half(3, specs4[2:4], [kT[:, ts(3, TB)], None], 4)

        # rots for blocks 0+1 slot between the first fill steps so the raw
        # copies complete first
        run_fill(2)
        rot_phase(raw_k0, 0, kT[:, ts(0, TB)])
        rot_phase(raw_q00, 0, qT[0][:, ts(0, TB)])
        rot_phase(raw_q10, 0, qT[1][:, ts(0, TB)])
        run_fill(2)
        rot_phase(raw_k1, 1, kT[:, ts(1, TB)])
        rot_phase(raw_q01, 1, qT[0][:, ts(1, TB)])
        rot_phase(raw_q11, 1, qT[1][:, ts(1, TB)])
        run_fill(4)

        # attention blocks interleave with the remaining proj fills, then with
        # c_proj items; gates make sure block bt's rope fills were emitted.
        attn_block(0, fill=0)
        attn_block(1, fill=2)
        drain_fill_until(2)          # kT/qT[2] rope emitted before attn2
        attn_block(2, fill=2)
        drain_fill_until(4)
        attn_block(3, fill=2)
        last_row[0] = cp_queue[-1][0]
        while fill_q:
            fill_q.pop(0)[1]()
        drain_cp = len(cp_queue)
        for _ in range(drain_cp):
            emit_cp_item(finale=True)


def build_nc():
    if "nc" in _NC_CACHE:
        return _NC_CACHE["nc"]
    nc = bacc.Bacc("TRN2", target_bir_lowering=False, debug=False, num_devices=NCORES)
    dr = {}

    def din(name, shape, dt):
        dr[name] = nc.dram_tensor(name, shape, dt, kind="ExternalInput").ap()

    din("xt", (128, NCT, T), BF16)
    din("wqkv", (128, NCT, 4 * HD), BF16)
    din("wo", (HD, HQ * C), BF16)
    din("trig", (HD, 2 * T), BF16)
    din("masks", (128, 128), F32)
    din("rmat", (HD, HD), BF16)
    out_d = nc.dram_tensor("out", (T, C), BF16, kind="ExternalOutput").ap()

    with tile.TileContext(nc) as tc:
        _emit(tc, dr, out_d)
    nc.compile()
    _NC_CACHE["nc"] = nc
    return nc


def build_nc_chain(reps):
    """Timing-only variant: the kernel body emitted `reps` times in one NEFF.
    (t_reps - t_1)/(reps-1) isolates steady-state per-iteration time from the
    ~2ms axon dispatch overhead."""
    nc = bacc.Bacc("TRN2", target_bir_lowering=False, debug=False, num_devices=NCORES)
    dr = {}

    def din(name, shape, dt):
        dr[name] = nc.dram_tensor(name, shape, dt, kind="ExternalInput").ap()

    din("xt", (128, NCT, T), BF16)
    din("wqkv", (128, NCT, 4 * HD), BF16)
    din("wo", (HD, HQ * C), BF16)
    din("trig", (HD, 2 * T), BF16)
    din("masks", (128, 128), F32)
    din("rmat", (HD, HD), BF16)
    out_d = nc.dram_tensor("out", (T, C), BF16, kind="ExternalOutput").ap()

    with tile.TileContext(nc) as tc:
        for _ in range(reps):
            _emit(tc, dr, out_d)
    nc.compile()
    return nc


def make_in_maps(x, Wq, Wk, Wv, Wo, position_ids):
    """Host-side sharding + constant tables. Returns one input dict per core."""
    x = np.asarray(x, dtype=np.float32)
    xt = _bf16(x.reshape(T, C).T)                      # [C, T]
    xt = np.ascontiguousarray(
        xt.reshape(NCT, 128, T).transpose(1, 0, 2))    # [128, NCT, T]

    pos = np.asarray(position_ids).astype(np.float64)  # [T]
    inv = 1.0 / (ROPE_BASE ** (np.arange(0, HD, 2, dtype=np.float64) / HD))
    fr = pos[:, None] * inv[None, :]                   # [T, 64]
    emb = np.concatenate([fr, fr], axis=-1)            # [T, 128]
    cost = _bf16(np.cos(emb).T)                        # [128, T]
    sint = _bf16(np.sin(emb).T)

    si = np.arange(128)[:, None]
    ti = np.arange(128)[None, :]
    masks = np.where(si > ti, NEG, 0.0).astype(np.float32)   # [128, 128] triangle

    # rotate_half operator: rot = R @ q  with  rot[d<64] = -q[d+64],
    # rot[d>=64] = q[d-64].  matmul computes lhsT.T @ rhs, so ship R.T.
    R = np.zeros((HD, HD), dtype=np.float32)
    R[np.arange(64), np.arange(64) + 64] = -1.0
    R[np.arange(64, 128), np.arange(64, 128) - 64] = 1.0
    rmat = _bf16(R.T)

    Wq = np.asarray(Wq, dtype=np.float32)
    Wk = np.asarray(Wk, dtype=np.float32)
    Wv = np.asarray(Wv, dtype=np.float32)
    Wo = np.asarray(Wo, dtype=np.float32)

    in_maps = []
    for i in range(NCORES):
        g = i // (NCORES // KH)                        # kv head for this core
        in_maps.append({
            "xt": xt,
            "wqkv": np.ascontiguousarray(_bf16(np.concatenate([
                Wq[:, i * HQ * HD:(i + 1) * HQ * HD],
                Wk[:, g * HD:(g + 1) * HD],
                Wv[:, g * HD:(g + 1) * HD]], axis=1))
                .reshape(NCT, 128, 4 * HD).transpose(1, 0, 2)),
            "wo": _bf16(np.concatenate([
                Wo[i * HQ * HD + h * HD:i * HQ * HD + (h + 1) * HD, :]
                for h in range(HQ)], axis=1)),
            "trig": np.concatenate([cost, sint], axis=1),
            "masks": masks,
            "rmat": rmat,
        })
    return in_maps


def run(inputs, trace=False):
    nc = build_nc()
    in_maps = make_in_maps(**inputs)
    res = run_bass_kernel_spmd(
        nc, in_maps, core_ids=list(range(NCORES)), trace=trace)
    out = np.zeros((T, C), dtype=np.float32)
    for i in range(NCORES):
        out += np.asarray(res.results[i]["out"], dtype=np.float32)
    return out.reshape(B, T, C), res


def kernel(x, Wq, Wk, Wv, Wo, position_ids):
    out, _ = run(dict(x=x, Wq=Wq, Wk=Wk, Wv=Wv, Wo=Wo,
                      position_ids=position_ids), trace=False)
    return out



# revision 39
# speedup vs baseline: 1.2548x; 1.2548x over previous
"""Tensor-parallel causal self-attention (GQA + RoPE) on one TRN2 chip (8 NeuronCores).

Megatron-style TP over heads: core i computes q-heads {2i, 2i+1} (kv head i//2),
runs blocked causal attention for those heads entirely on-core, then the partial
c_proj  y_i @ Wo[rows_i, :].  The 8 partial [T, C] outputs are summed on the host
(the TP all-reduce), which is pure gather/unshard data movement.

Layout (contraction dims on SBUF partitions):
  xT   [128, NCT, T] tile-major (host pre-arranged, bf16) so wide strided DMAs
      stay legal; first halves stream per-tile ahead of the 8-chain group.
  qT/kT = Wq/Wk-proj emitted directly as [HD, T] via lhsT=W, rhs=xT
  RoPE rotate_half runs on PE as a +-1 permutation matmul; split into raw-copy
      (ACT/DVE) and rot-matmul phases so chain PSUM frees early.
  scoresT [s,t]: two 128-s-tiles per [128,1024] PSUM strip (2 banks); ONE wide
      exp per strip on ACT (amortizes activation overhead); diagonal strips
      exp only the live causal ranges.
  softmax denominator: DVE accumulates pt strips elementwise in bf16, then
      per-(h,t-block) gpsimd partition_all_reduce (reduce+broadcast in one op),
      pipelined in halves with the DVE reciprocal to cut block-boundary
      latency; normalization folds into the PSUM->SBUF eviction multiply.
      No PE cycles are spent on the denominator.
  v natural [s,d] from a vT projection + DMA-transpose (N=512 matmuls)
  outT [d,t] += pv accumulation with lhsT=v_nat, rhs=pt strip slices
  c_proj (m,n) items drain from a work-queue interleaved into attention blocks
      to fill PE stalls while ACT runs exp; evictions alternate ACT/DVE
      (Pool cannot touch PSUM); finale items rotate through the idle score
      strips' PSUM banks; the last row DMAs out in chunks for a short tail.

Schedule: 8-chain ci-interleaved group (t-blocks 0+1) races the input DMA
stream; t-blocks 2+3 project as half-blocks from a PE fill queue interleaved
between attention strips, so attn0/attn1's exp/softmax work rides inside the
projection phase.  Causal masking: off-diagonal s-tiles need no mask; the 4
diagonal s-tiles per t-block are computed at narrowed width with a -1e30
additive mask for the intra-tile triangle.
"""

import math
from contextlib import ExitStack

import ml_dtypes
import numpy as np

import concourse.bass as bass
import concourse.tile as tile
from concourse import bacc, mybir, bass_isa
from concourse.bass import ts, ds
from concourse.bass_utils import run_bass_kernel_spmd

# ---------------- problem constants (hardcoded per contest rules) ------------
B, T, C = 1, 2048, 2048
H, KH, HD = 16, 4, 128
NCORES = 8
HQ = H // NCORES            # 2 query heads per core
ROPE_BASE = 10000.0
SCALE = 1.0 / math.sqrt(HD)
TB = 512                    # t-block (moving free dim) for attention
NT = T // TB                # 4
NCT = C // 128              # 16 contraction tiles for projections
NCH = NCT // 2              # 8: ci tiles for the halved v-projection partials
PAIRS = [[0, 1], [2, 3], [4, 5], [6, 7]]
NS = T // 128               # 16 key/value s-tiles
BF16 = mybir.dt.bfloat16
F32 = mybir.dt.float32
EXPF = mybir.ActivationFunctionType.Exp
NEG = -1.0e30

_NC_CACHE = {}


def _bf16(a):
    return np.ascontiguousarray(np.asarray(a, dtype=np.float32).astype(ml_dtypes.bfloat16))


def _emit(tc, dr, out_d, cc, cc_insts):
    nc = tc.nc
    with ExitStack() as ctx:
        def sb(name, bufs):
            return ctx.enter_context(tc.tile_pool(name=name, bufs=bufs))

        def ps(name, bufs):
            return ctx.enter_context(tc.tile_pool(name=name, bufs=bufs, space="PSUM"))

        p_xt = sb("xt", 1)
        p_wq = sb("wq", 1)
        p_wo = sb("wo", 1)
        p_trig = sb("trig", 1)
        p_mask = sb("mask", 1)
        p_junk = sb("junk", 1)
        p_qt = sb("qt", HQ)
        p_kt = sb("kt", 1)
        p_v = sb("v", NS)
        p_vh = sb("vh", 4)     # v-partial bf16 staging [128,512]
        p_vf = sb("vf", 1)     # AllReduced vT [128, T]
        p_yt = sb("yt", HQ)
        p_qraw = sb("qraw", 8)
        p_rtmp = sb("rtmp", 6)
        p_pt = sb("pt", 7)          # [128,1024] bf16 exp strips
        p_acc = sb("acc", 3)        # [128,TB] bf16 denominator partial sums
        p_rb = sb("rb", 4)          # [128,TB] f32 partition_all_reduce out
        p_ri = sb("ri", 3)          # [128,TB] f32 reciprocal out
        p_stage = sb("stage", 6)
        ps_strip = ps("ps_strip", 2)  # [128,1024] score strips (+rope rot)
        ps_acc = ps("ps_acc", 2)      # [128,TB] attention outT accumulators
        ps_cp = ps("ps_cp", 2)        # [128,TB] c_proj / proj k,v chains

        # ---------------- input loads (HWDGE queues: sync + scalar only; the
        # gpsimd queue is software-DGE and burns ~1us of Pool engine time per
        # descriptor, delaying rope adds / evictions) -------------------------
        wqkv_all = p_wq.tile([128, NCT, 4 * HD], BF16, name="wqkv", tag="wq")
        xt_all = p_xt.tile([128, NCT, T], BF16, name="xta", tag="xt")
        xt = [xt_all[:, i, :] for i in range(NCT)]
        # The 8-chain group (t-blocks 0+1) only reads xt[:, 0:1024]: stream all
        # first-halves at the head of the sync queue (delivery ~1.1us/tile vs
        # ~1.7us/tile consumption); second halves land as one strided DMA
        # behind them (merged: the serial HWDGE descriptor processor is near
        # saturation during the input phase).
        nc.scalar.dma_start(wqkv_all[:, 0, :], dr["wqkv"][:, 0, :])
        nc.sync.dma_start(xt_all[:, 0, 0:512], dr["xt"][:, 0, 0:512])
        nc.sync.dma_start(xt_all[:, 0, 512:1024], dr["xt"][:, 0, 512:1024])
        nc.sync.dma_start(xt_all[:, 1, 0:1024], dr["xt"][:, 1, 0:1024])
        for i in range(2, NCT, 2):
            nc.sync.dma_start(xt_all[:, i:i + 2, 0:1024],
                              dr["xt"][:, i:i + 2, 0:1024])
        for i in range(0, NCT, 2):   # second halves: 2-tile chunks
            nc.sync.dma_start(xt_all[:, i:i + 2, 1024:T],
                              dr["xt"][:, i:i + 2, 1024:T])
        nc.scalar.dma_start(wqkv_all[:, 1, :], dr["wqkv"][:, 1, :])
        nc.scalar.dma_start(wqkv_all[:, 2:4, :], dr["wqkv"][:, 2:4, :])
        nc.scalar.dma_start(wqkv_all[:, 4:8, :], dr["wqkv"][:, 4:8, :])
        wq = [wqkv_all[:, i, :] for i in range(NCT)]  # cols [0, 256) = Wq
        wk = [wqkv_all[:, i, 256:384] for i in range(NCT)]
        wv = [wqkv_all[:, i, 384:512] for i in range(NCT)]
        # trig first-halves + rot/mask tables land mid-stream (needed by the
        # grp rope evictions at ~31us); wqkv tail keeps pace with the group.
        trig = p_trig.tile([128, 2 * T], BF16, name="trig", tag="trig")
        cost = trig[:, 0:T]
        sint = trig[:, T:2 * T]
        nc.scalar.dma_start(trig[:, 0:1024], dr["trig"][:, 0:1024])
        nc.scalar.dma_start(trig[:, T:T + 1024], dr["trig"][:, T:T + 1024])
        rmat = p_mask.tile([128, 128], BF16, name="rmat", tag="rmat")
        nc.scalar.dma_start(rmat[:], dr["rmat"][:, :])
        masks = p_mask.tile([128, 128], F32, name="masks", tag="mask")
        nc.scalar.dma_start(masks[:], dr["masks"][:, :])
        nc.scalar.dma_start(wqkv_all[:, 8:10, :], dr["wqkv"][:, 8:10, :])
        nc.scalar.dma_start(wqkv_all[:, 10:12, :], dr["wqkv"][:, 10:12, :])
        nc.scalar.dma_start(wqkv_all[:, 12:14, :], dr["wqkv"][:, 12:14, :])
        nc.scalar.dma_start(wqkv_all[:, 14:NCT, :], dr["wqkv"][:, 14:NCT, :])
        nc.scalar.dma_start(trig[:, 1024:T], dr["trig"][:, 1024:T])
        nc.scalar.dma_start(trig[:, T + 1024:2 * T], dr["trig"][:, T + 1024:2 * T])
        wo2 = p_wo.tile([128, HQ * C], BF16, name="wo2", tag="wo")
        wo = [wo2[:, ts(h, C)] for h in range(HQ)]
        nc.scalar.dma_start(wo2[:, 0:C], dr["wo"][:, 0:C])
        nc.scalar.dma_start(wo2[:, C:2 * C], dr["wo"][:, C:2 * C])
        # preload exp table while DMAs run (after the scalar-queue dma issues
        # so LoadActFuncSet doesn't block them)
        junk = p_junk.tile([128, 512], BF16, name="junk", tag="junk")
        nc.vector.memset(junk[0:1, 0:1], 0.0)
        nc.scalar.activation(junk[0:1, 0:1], junk[0:1, 0:1], EXPF, scale=1.0)

        # ---------------- rope helpers (two-phase) ----------------
        def raw_phase(psum, eng=None):
            """chain psum [128(d), TB] f32 -> raw bf16 SBUF copy (ACT/DVE)."""
            raw = p_qraw.tile([128, TB], BF16, name="rraw", tag="qraw")
            if eng is nc.vector:
                nc.vector.tensor_copy(raw[:], psum[:])
            else:
                nc.scalar.copy(raw[:], psum[:])
            return raw

        rope_add_rr = [0]

        def rot_phase(raw, bt, dst):
            """raw [128, TB] bf16 -> RoPE -> dst (bf16 slice [128, TB])."""
            cs = cost[:, ts(bt, TB)]
            sn = sint[:, ts(bt, TB)]
            rot = ps_strip.tile([128, TB], F32, name="rot", tag="strip")
            nc.tensor.matmul(rot[:], lhsT=rmat[:], rhs=raw[:], start=True, stop=True)
            t1 = p_rtmp.tile([128, TB], BF16, name="rt1", tag="rtmp")
            t2 = p_rtmp.tile([128, TB], BF16, name="rt2", tag="rtmp")
            nc.gpsimd.tensor_mul(t1[:], raw[:], cs)
            nc.vector.tensor_mul(t2[:], rot[:], sn)
            rope_add_rr[0] += 1
            nc.vector.tensor_add(dst, t1[:], t2[:])

        qT = [p_qt.tile([128, T], BF16, name=f"qT{h}", tag="qt") for h in range(HQ)]
        kT = p_kt.tile([128, T], BF16, name="kT", tag="kt")
        v = [p_v.tile([128, HD], BF16, name=f"v{s}", tag="v") for s in range(NS)]
        yT = [p_yt.tile([128, T], BF16, name=f"yT{h}", tag="yt") for h in range(HQ)]
        dma_rr = [nc.sync, nc.scalar]

        vfull = p_vf.tile([128, T], BF16, name="vfull", tag="vf")

        def ar(nm):
            inst = nc.gpsimd.collective_compute(
                "AllReduce", mybir.AluOpType.add, replica_groups=PAIRS,
                ins=[cc[nm + "_in"][:, :]], outs=[cc[nm + "_out"][:, :]])
            cc_insts.append((nm, inst))

        def v_evict(pv, bt):
            """v partial (half contraction) -> bf16 -> pairwise AllReduce ->
            vfull slice -> natural v s-tiles via DMA transpose."""
            nm = f"v{bt}"
            vh = p_vh.tile([128, TB], BF16, name="vh", tag="vh")
            nc.vector.tensor_copy(vh[:], pv[:])
            nc.scalar.dma_start(cc[nm + "_in"][:, :], vh[:])
            ar(nm)
            nc.sync.dma_start(vfull[:, ts(bt, TB)], cc[nm + "_out"][:, :])
            for j in range(TB // 128):
                nc.sync.dma_start_transpose(v[4 * bt + j][:],
                                            vfull[:, ds(bt * TB + j * 128, 128)])

        # ---------------- c_proj work queue ----------------
        cp_queue = []          # (m, n) items ready to emit
        st_tiles = {}
        evict_rr = [0]
        last_row = [-1]
        fin_strip = [None]

        def emit_cp_item(finale=False):
            if not cp_queue:
                return False
            m, n = cp_queue.pop(0)
            if n == 0:
                st_tiles[m] = p_stage.tile([128, C], BF16, name="st", tag="stage")
            st = st_tiles[m]
            if finale and m == last_row[0]:
                # last row: attention's accumulator banks are idle by now --
                # rotating through them pipelines the final items 2 deeper
                pc = ps_acc.tile([128, TB], F32, name="pl", tag="acc")[:]
            elif finale and evict_rr[0] % 2 == 0:
                # strips are idle in the finale: independent [128,512] tiles
                # (per-TILE PSUM dep tracking would serialize shared pairs)
                pc = ps_strip.tile([128, TB], F32, name="fs", tag="strip")[:]
            else:
                pc = ps_cp.tile([128, TB], F32, name="pc", tag="cp")
            for h in range(HQ):
                nc.tensor.matmul(
                    pc[:], lhsT=yT[h][:, ts(m, 128)], rhs=wo[h][:, ts(n, TB)],
                    start=(h == 0), stop=(h == HQ - 1))
            e = (evict_rr[0] + 1) % 2   # Pool/GpSimd cannot read PSUM
            evict_rr[0] += 1
            if m == last_row[0]:
                e = 1             # last row: DVE (ACT SEQ is issuing out DMAs)
            if e == 0:
                nc.scalar.copy(st[:, ts(n, TB)], pc[:])
            else:
                nc.vector.tensor_copy(st[:, ts(n, TB)], pc[:])
            if m == last_row[0]:        # last row: chunked DMA, short tail
                nc.scalar.dma_start(out_d[ts(m, 128), ts(n, TB)],
                                    st[:, ts(n, TB)])
            if finale and n % 2 == 1:    # finale rows: half-row DMAs issue
                dma_rr[m % 2].dma_start(   # earlier, evening out the wire
                    out_d[ts(m, 128), ds((n - 1) * TB, 2 * TB)],
                    st[:, ds((n - 1) * TB, 2 * TB)])
            elif n == 3:
                dma_rr[m % 2].dma_start(out_d[ts(m, 128), 0:C], st[:, 0:C])
                dma_rr[(m + 1) % 2].dma_start(out_d[ts(m, 128), C:2 * C],
                                              st[:, C:2 * C])
            return True

        # ---------------- PE fill queue (proj half-blocks, then cp items) ----
        fill_q = []            # (tag, thunk)

        def run_fill(k):
            while k > 0 and fill_q:
                fill_q.pop(0)[1]()
                k -= 1
            while k > 0 and emit_cp_item():
                k -= 1

        def drain_fill_until(tag):
            while fill_q and fill_q[0][0] <= tag:
                fill_q.pop(0)[1]()

        # ---------------- attention ----------------
        def attn_block(bt, fill=0):
            nbs = 4 * (bt + 1)
            for h in range(HQ):
                po = ps_acc.tile([128, TB], F32, name="po", tag="acc")
                acc = p_acc.tile([128, TB], BF16, name="dacc", tag="acc")
                prevs = []
                for k in range(nbs // 2):
                    b0, b1 = 2 * k, 2 * k + 1
                    strip = ps_strip.tile([128, 1024], F32, name="sc", tag="strip")
                    pt = p_pt.tile([128, 1024], BF16, name="pt", tag="pt")
                    offs = []
                    for si, bs in enumerate((b0, b1)):
                        j = bs - 4 * bt
                        off = max(j, 0) * 128
                        offs.append(off)
                        nc.tensor.matmul(
                            strip[:, 512 * si + off:512 * (si + 1)],
                            lhsT=kT[:, ts(bs, 128)],
                            rhs=qT[h][:, ds(bt * TB + off, TB - off)],
                            start=True, stop=True)
                        if j >= 0:   # diagonal: intra-tile causal triangle
                            nc.vector.tensor_add(
                                strip[:, ds(512 * si + off, 128)],
                                strip[:, ds(512 * si + off, 128)],
                                masks[:, :])
                    if offs[1] == 0:
                        nc.scalar.activation(pt[:], strip[:], EXPF, scale=SCALE)
                    elif offs == [0, 128]:
                        # small dead gap: ONE wide exp (cols 512:640 are dead
                        # but never read) beats two calls' overhead
                        nc.scalar.activation(pt[:], strip[:], EXPF, scale=SCALE)
                    else:   # diagonal strips: exp only the live ranges
                        nc.scalar.activation(pt[:, offs[0]:512],
                                             strip[:, offs[0]:512],
                                             EXPF, scale=SCALE)
                        nc.scalar.activation(pt[:, 512 + offs[1]:1024],
                                             strip[:, 512 + offs[1]:1024],
                                             EXPF, scale=SCALE)
                    # denominator partial sums (DVE, partition lanes)
                    if k == 0:
                        nc.vector.tensor_copy(acc[:], pt[:, 0:512])
                    else:
                        nc.vector.tensor_add(
                            acc[:, offs[0]:512], acc[:, offs[0]:512],
                            pt[:, offs[0]:512])
                    nc.gpsimd.tensor_add(
                        acc[:, offs[1]:512], acc[:, offs[1]:512],
                        pt[:, 512 + offs[1]:1024])
                    run_fill(fill)
                    if len(prevs) >= 2:
                        pv_emit(po, bt, nbs, *prevs.pop(0))
                    prevs.append((pt, b0, b1, offs))
                for pv_args in prevs:
                    pv_emit(po, bt, nbs, *pv_args)
                # normalize: partition all-reduce -> reciprocal -> fold into
                # the PSUM->SBUF eviction multiply
                rb = p_rb.tile([128, TB], F32, name="rb", tag="rb")
                ri = p_ri.tile([128, TB], F32, name="ri", tag="ri")
                for half in range(2):   # halves pipeline Pool->DVE: shorter
                    hs = ds(half * 256, 256)        # block-boundary latency
                    nc.gpsimd.partition_all_reduce(
                        rb[:, hs], acc[:, hs], channels=128,
                        reduce_op=bass_isa.ReduceOp.add)
                    nc.vector.reciprocal(ri[:, hs], rb[:, hs])
                    for e in range(2):
                        c = ds(bt * TB + half * 256 + e * 128, 128)
                        cp = ds(half * 256 + e * 128, 128)
                        nc.vector.tensor_mul(yT[h][:, c], po[:, cp], ri[:, cp])
            cp_queue.extend((4 * bt + sub, n)
                            for sub in range(NT) for n in range(C // TB))

        def pv_emit(po, bt, nbs, pt, b0, b1, offs):
            for si, bs in enumerate((b0, b1)):
                off = offs[si]
                nc.tensor.matmul(
                    po[:, off:TB], lhsT=v[bs][:],
                    rhs=pt[:, 512 * si + off:512 * (si + 1)],
                    start=(bs == 0), stop=(bs == nbs - 1))

        # -------- projections --------
        # t-blocks 0+1: one ci-interleaved 8-chain group across all 8 PSUM
        # banks — maximizes PE progress while x is still streaming in.
        s0 = ps_strip.tile([128, 1024], F32, name="gs0", tag="strip")
        s1 = ps_strip.tile([128, 1024], F32, name="gs1", tag="strip")
        a0 = ps_acc.tile([128, TB], F32, name="ga0", tag="acc")
        a1 = ps_acc.tile([128, TB], F32, name="ga1", tag="acc")
        c0 = ps_cp.tile([128, TB], F32, name="gc0", tag="cp")
        c1 = ps_cp.tile([128, TB], F32, name="gc1", tag="cp")
        grp = [s0[:, 0:512], s0[:, 512:1024], s1[:, 0:512], s1[:, 512:1024],
               a0[:], a1[:], c0[:], c1[:]]
        specs4 = [(wq, 0), (wq, 1), (wk, None), (wv, None)]
        for ci in range(NCT):
            for i in range(8):
                b = i // 4
                w, hh = specs4[i % 4]
                if w is wv and ci >= NCH:
                    continue           # v partials: half contraction
                lhsT = w[ci][:, ts(hh, HD)] if hh is not None else w[ci]
                nc.tensor.matmul(
                    grp[i], lhsT=lhsT, rhs=xt[ci][:, ts(b, TB)],
                    start=(ci == 0),
                    stop=(ci == (NCH - 1 if w is wv else NCT - 1)))
        # rope raw copies split ACT/DVE (k first: attn0 needs kT earliest);
        # chain PSUM frees as soon as its raw lands.
        raw_k0 = raw_phase(grp[2], nc.vector)
        raw_q00 = raw_phase(grp[0], nc.scalar)
        raw_q10 = raw_phase(grp[1], nc.vector)
        v_evict(grp[3], 0)
        raw_k1 = raw_phase(grp[6], nc.scalar)
        raw_q01 = raw_phase(grp[4], nc.vector)
        raw_q11 = raw_phase(grp[5], nc.scalar)
        v_evict(grp[7], 1)

        # proj half-blocks for t-blocks 2+3 go into the PE fill queue
        def make_proj_half(bt, kinds, dsts, tag):
            box = {}

            def step(ci):
                def thunk():
                    if "tiles" not in box:
                        box["tiles"] = [ps_cp.tile([128, TB], F32, name="ph",
                                                   tag="cp") for _ in kinds]
                    for t, (w, hh) in zip(box["tiles"], kinds):
                        if w is wv and ci >= NCH:
                            continue   # v partials: half contraction
                        lhsT = w[ci][:, ts(hh, HD)] if hh is not None else w[ci]

                        nc.tensor.matmul(
                            t[:], lhsT=lhsT, rhs=xt[ci][:, ts(bt, TB)],
                            start=(ci == 0),
                            stop=(ci == (NCH - 1 if w is wv else NCT - 1)))
                return thunk

            def finisher():
                engs = [nc.scalar, nc.vector]
                for i, (t, dst) in enumerate(zip(box["tiles"], dsts)):
                    if dst is None:
                        v_evict(t, bt)
                    else:
                        raw = raw_phase(t[:], engs[i % 2])
                        rot_phase(raw, bt, dst)

            for ci in range(NCT):
                fill_q.append((tag, step(ci)))
            fill_q.append((tag, finisher))

        make_proj_half(2, specs4[0:2], [qT[0][:, ts(2, TB)], qT[1][:, ts(2, TB)]], 1)
        make_proj_half(2, specs4[2:4], [kT[:, ts(2, TB)], None], 2)
        make_proj_half(3, specs4[0:2], [qT[0][:, ts(3, TB)], qT[1][:, ts(3, TB)]], 3)
        make_proj_half(3, specs4[2:4], [kT[:, ts(3, TB)], None], 4)

        # rots for blocks 0+1 slot between the first fill steps so the raw
        # copies complete first
        rot_phase(raw_k0, 0, kT[:, ts(0, TB)])
        rot_phase(raw_q00, 0, qT[0][:, ts(0, TB)])
        rot_phase(raw_q10, 0, qT[1][:, ts(0, TB)])
        rot_phase(raw_k1, 1, kT[:, ts(1, TB)])
        rot_phase(raw_q01, 1, qT[0][:, ts(1, TB)])
        rot_phase(raw_q11, 1, qT[1][:, ts(1, TB)])
        run_fill(8)

        # attention blocks interleave with the remaining proj fills, then with
        # c_proj items; gates make sure block bt's rope fills were emitted.
        attn_block(0, fill=1)
        attn_block(1, fill=1)
        drain_fill_until(2)          # kT/qT[2] rope emitted before attn2
        attn_block(2, fill=2)
        drain_fill_until(4)
        attn_block(3, fill=3)
        last_row[0] = cp_queue[-1][0]
        while fill_q:
            fill_q.pop(0)[1]()
        drain_cp = len(cp_queue)
        for _ in range(drain_cp):
            emit_cp_item(finale=True)


def build_nc():
    if "nc" in _NC_CACHE:
        return _NC_CACHE["nc"]
    nc = bacc.Bacc("TRN2", target_bir_lowering=False, debug=False, num_devices=NCORES)
    dr = {}

    def din(name, shape, dt):
        dr[name] = nc.dram_tensor(name, shape, dt, kind="ExternalInput").ap()

    din("xt", (128, NCT, T), BF16)
    din("wqkv", (128, NCT, 4 * HD), BF16)
    din("wo", (HD, HQ * C), BF16)
    din("trig", (HD, 2 * T), BF16)
    din("masks", (128, 128), F32)
    din("rmat", (HD, HD), BF16)
    out_d = nc.dram_tensor("out", (T, C), BF16, kind="ExternalOutput").ap()

    with tile.TileContext(nc) as tc:
        _emit(tc, dr, out_d)
    nc.compile()
    _NC_CACHE["nc"] = nc
    return nc


def build_nc_chain(reps):
    """Timing-only variant: the kernel body emitted `reps` times in one NEFF.
    (t_reps - t_1)/(reps-1) isolates steady-state per-iteration time from the
    ~2ms axon dispatch overhead."""
    nc = bacc.Bacc("TRN2", target_bir_lowering=False, debug=False, num_devices=NCORES)
    dr = {}

    def din(name, shape, dt):
        dr[name] = nc.dram_tensor(name, shape, dt, kind="ExternalInput").ap()

    din("xt", (128, NCT, T), BF16)
    din("wqkv", (128, NCT, 4 * HD), BF16)
    din("wo", (HD, HQ * C), BF16)
    din("trig", (HD, 2 * T), BF16)
    din("masks", (128, 128), F32)
    din("rmat", (HD, HD), BF16)
    out_d = nc.dram_tensor("out", (T, C), BF16, kind="ExternalOutput").ap()

    with tile.TileContext(nc) as tc:
        for _ in range(reps):
            _emit(tc, dr, out_d)
    nc.compile()
    return nc


def make_in_maps(x, Wq, Wk, Wv, Wo, position_ids):
    """Host-side sharding + constant tables. Returns one input dict per core."""
    x = np.asarray(x, dtype=np.float32)
    xt = _bf16(x.reshape(T, C).T)                      # [C, T]
    xt = np.ascontiguousarray(
        xt.reshape(NCT, 128, T).transpose(1, 0, 2))    # [128, NCT, T]

    pos = np.asarray(position_ids).astype(np.float64)  # [T]
    inv = 1.0 / (ROPE_BASE ** (np.arange(0, HD, 2, dtype=np.float64) / HD))
    fr = pos[:, None] * inv[None, :]                   # [T, 64]
    emb = np.concatenate([fr, fr], axis=-1)            # [T, 128]
    cost = _bf16(np.cos(emb).T)                        # [128, T]
    sint = _bf16(np.sin(emb).T)

    si = np.arange(128)[:, None]
    ti = np.arange(128)[None, :]
    masks = np.where(si > ti, NEG, 0.0).astype(np.float32)   # [128, 128] triangle

    # rotate_half operator: rot = R @ q  with  rot[d<64] = -q[d+64],
    # rot[d>=64] = q[d-64].  matmul computes lhsT.T @ rhs, so ship R.T.
    R = np.zeros((HD, HD), dtype=np.float32)
    R[np.arange(64), np.arange(64) + 64] = -1.0
    R[np.arange(64, 128), np.arange(64, 128) - 64] = 1.0
    rmat = _bf16(R.T)

    Wq = np.asarray(Wq, dtype=np.float32)
    Wk = np.asarray(Wk, dtype=np.float32)
    Wv = np.asarray(Wv, dtype=np.float32)
    Wo = np.asarray(Wo, dtype=np.float32)

    in_maps = []
    for i in range(NCORES):
        g = i // (NCORES // KH)                        # kv head for this core
        in_maps.append({
            "xt": xt,
            "wqkv": np.ascontiguousarray(_bf16(np.concatenate([
                Wq[:, i * HQ * HD:(i + 1) * HQ * HD],
                Wk[:, g * HD:(g + 1) * HD],
                Wv[:, g * HD:(g + 1) * HD]], axis=1))
                .reshape(NCT, 128, 4 * HD).transpose(1, 0, 2)),
            "wo": _bf16(np.concatenate([
                Wo[i * HQ * HD + h * HD:i * HQ * HD + (h + 1) * HD, :]
                for h in range(HQ)], axis=1)),
            "trig": np.concatenate([cost, sint], axis=1),
            "masks": masks,
            "rmat": rmat,
        })
    return in_maps


def run(inputs, trace=False):
    nc = build_nc()
    in_maps = make_in_maps(**inputs)
    res = run_bass_kernel_spmd(
        nc, in_maps, core_ids=list(range(NCORES)), trace=trace)
    out = np.zeros((T, C), dtype=np.float32)
    for i in range(NCORES):
        out += np.asarray(res.results[i]["out"], dtype=np.float32)
    return out.reshape(B, T, C), res


def kernel(x, Wq, Wk, Wv, Wo, position_ids):
    out, _ = run(dict(x=x, Wq=Wq, Wk=Wk, Wv=Wv, Wo=Wo,
                      position_ids=position_ids), trace=False)
    return out

